# revision 25
# baseline (speedup 1.0000x reference)
"""Causal self-attention (B=2, T=2048, C=1024, H=16) on 8 trn2 NeuronCores.

Sharding: core c = (b, g) with b = c // 4 (batch), g = c % 4 (head-group of 4
heads = 256 dims). Per core:
  1. Q/K projection from x[b].T (bf16 weights, fp32r outputs, bias fused
     into DVE copies): Q^T, K^T in [d, t] layout (head-pair tiles).
  2. t4-interleaved main loop (t4 = 512-token chunk, ascending):
     V chunk t4 ([t, d] layout, ones column per head so softmax denominators
     fall out of the AV matmul), then flash-style attention for q-chunk t4
     in S^T = K Q^T layout: S^T psum (pipelined one k-tile ahead) -> exp
     (ACT, 1/8 scale fused) -> causal mask on the diagonal 128-block ->
     AV accumulation with [V | 1] stationary.  Normalization at chunk end
     (row 64 = denominator, reciprocal_approx_fast + partition broadcast).
  3. One merged 4-core AllGather per chunk ([256, 512] -> [1024, 512],
     both head-pairs) pipelined behind later chunks' attention.
  4. Output projection column-sharded, emitted two chunks behind its
     AllGather so the PE queue never blocks on the collective.
"""
import math

import numpy as np
import ml_dtypes

B, T, C, H = 2, 2048, 1024, 16
HD = C // H          # 64 head dim
G = 4                # head-groups (cores per batch)
HPG = H // G         # 4 heads per group
DG = HPG * HD        # 256 dims per group
N_CORES = 8
KC = C // 128        # 8 contraction chunks
NKT = T // 128       # 16 k-tiles
NQC = T // 512       # 4 q-chunks in attention
RG = [[0, 1, 2, 3], [4, 5, 6, 7]]

_NC_CACHE = {}


def _build():
    import concourse.bacc as bacc
    import concourse.mybir as mybir
    import concourse.tile as tile

    f32 = mybir.dt.float32
    f32r = mybir.dt.float32r
    bf16 = mybir.dt.bfloat16
    Exp = mybir.ActivationFunctionType.Exp

    nc = bacc.Bacc("TRN2", num_devices=N_CORES)

    xT_d = nc.dram_tensor("xT", [C, T], bf16, kind="ExternalInput")
    wq_d = nc.dram_tensor("wq", [C, DG], bf16, kind="ExternalInput")
    wk_d = nc.dram_tensor("wk", [C, DG], bf16, kind="ExternalInput")
    wv_d = nc.dram_tensor("wv", [C, DG], bf16, kind="ExternalInput")
    bq_d = nc.dram_tensor("bq", [2, 128, 1], f32, kind="ExternalInput")
    bk_d = nc.dram_tensor("bk", [2, 128, 1], f32, kind="ExternalInput")
    bv_d = nc.dram_tensor("bv", [1, DG], f32, kind="ExternalInput")
    # w_proj^T rows ordered to match the AllGather row layout
    wp_d = nc.dram_tensor("wpTa", [C, DG], bf16, kind="ExternalInput")
    bp_d = nc.dram_tensor("bp", [2, 128, 1], f32, kind="ExternalInput")
    mask_d = nc.dram_tensor("mask", [128, 128], bf16, kind="ExternalInput")
    ones_d = nc.dram_tensor("ones4", [128, HPG, 1], bf16, kind="ExternalInput")
    oT_d = nc.dram_tensor("oT", [DG, T], f32, kind="ExternalOutput")

    def dma_chunked(dst, src, n):
        w = dst.shape[-1]
        step = w // n
        for i in range(n):
            nc.sync.dma_start(dst[..., step * i:step * (i + 1)],
                              src[..., step * i:step * (i + 1)])

    with tile.TileContext(nc) as tc:
        with (
            tc.tile_pool(name="persist", bufs=1) as persist,
            tc.tile_pool(name="dram", bufs=1, space="DRAM") as dram,
        ):
            # ---- persistent SBUF ----
            QT = [persist.tile([128, T], bf16, name=f"qt{p}") for p in range(2)]
            KT = [persist.tile([128, T], bf16, name=f"kt{p}") for p in range(2)]
            V1 = [persist.tile([128, HPG * (HD + 2)], bf16, name=f"v{m}")
                  for m in range(NKT)]
            xT_sb = [persist.tile([128, T], bf16, name=f"x{k}")
                     for k in range(KC)]
            wq_sb = [persist.tile([128, DG], bf16, name=f"wq{k}")
                     for k in range(KC)]
            wk_sb = [persist.tile([128, DG], bf16, name=f"wk{k}")
                     for k in range(KC)]
            wv_sb = [persist.tile([128, DG], bf16, name=f"wv{k}")
                     for k in range(KC)]
            wpT_sb = [persist.tile([128, DG], bf16, name=f"wp_{k}")
                      for k in range(KC)]
            mask_sb = persist.tile([128, 128], bf16, name="mask_sb")
            bq_sb = [persist.tile([128, 1], f32, name=f"bq{j}") for j in range(2)]
            bk_sb = [persist.tile([128, 1], f32, name=f"bk{j}") for j in range(2)]
            bp_sb = [persist.tile([128, 1], f32, name=f"bp{j}") for j in range(2)]
            bv_row = persist.tile([1, DG], f32, name="bv_row")
            bv_bc = persist.tile([128, DG], f32, name="bv_bc")

            # per-chunk collective buffers (4-core ring AllGather, both
            # head-pairs merged: rows p*128..p*128+128)
            yq_in = [dram.tile([256, 512], bf16, name=f"yqi{cq}")
                     for cq in range(NQC)]
            yq_out = [dram.tile([1024, 512], bf16, name=f"yqo{cq}")
                      for cq in range(NQC)]
            # last chunk gathers per head-pair so the p0 AllGather runs
            # during p1's attention and the tail only waits on p1's
            yq3_in = [dram.tile([128, 512], bf16, name=f"yq3i{p}")
                      for p in range(2)]
            yq3_out = [dram.tile([512, 512], bf16, name=f"yq3o{p}")
                       for p in range(2)]

            # DMA issue order: tiny tensors first (the bv broadcast is the
            # first GpSimd op — a late bv blocks the whole GpSimd FIFO,
            # including every collective trigger), then x chunks (Q is
            # paced by them) interleaved with the weights
            nc.sync.dma_start(bv_row[:], bv_d[:])
            nc.sync.dma_start(mask_sb[:], mask_d[:])
            for j in range(2):
                nc.sync.dma_start(bq_sb[j][:], bq_d[j])
                nc.sync.dma_start(bk_sb[j][:], bk_d[j])
                nc.sync.dma_start(bp_sb[j][:], bp_d[j])
            nc.gpsimd.partition_broadcast(bv_bc[:], bv_row[:])
            dma_chunked(xT_sb[0], xT_d[0:128, :], 2)
            for k in range(KC):
                nc.sync.dma_start(wq_sb[k][:], wq_d[128 * k:128 * (k + 1), :])
            for k in range(1, KC):
                dma_chunked(xT_sb[k], xT_d[128 * k:128 * (k + 1), :], 2)
            for k in range(KC):
                nc.sync.dma_start(wk_sb[k][:], wk_d[128 * k:128 * (k + 1), :])
            for k in range(KC):
                nc.sync.dma_start(wv_sb[k][:], wv_d[128 * k:128 * (k + 1), :])
            for k in range(KC):
                nc.sync.dma_start(wpT_sb[k][:], wp_d[128 * k:128 * (k + 1), :])

            # ================= phase 1: Q projection =================
            with (
                tc.tile_pool(name="wup", bufs=1) as wup,
                tc.tile_pool(name="qkvps", bufs=1, space="PSUM") as qkvps,
            ):
                # PE warmup: dummy matmuls so HAM reaches K=8/8 before the
                # real (DMA-paced) matmuls arrive
                wu_a = wup.tile([128, 128], bf16, name="wu_a")
                wu_b = wup.tile([128, 512], bf16, name="wu_b")
                nc.vector.memset(wu_a[:], 0.5)
                nc.vector.memset(wu_b[:], 0.5)
                wu_ps = qkvps.tile([128, 512], f32, tag="qkvps", bufs=8,
                                   name="wu_ps")
                for _ in range(28):
                    nc.tensor.matmul(wu_ps[:], wu_a[:], wu_b[:],
                                     start=True, stop=True)

                # Q: psum [2 jh][4 t4] accumulated over kc (kc outer so the
                # matmuls start as soon as each x chunk lands)
                ps = [[qkvps.tile([128, 512], f32, tag="qkvps", bufs=8,
                                  name=f"psq_{jh}_{t4}")
                       for t4 in range(4)] for jh in range(2)]
                for kc in range(KC):
                    for jh in range(2):
                        for t4 in range(4):
                            nc.tensor.matmul(
                                ps[jh][t4][:],
                                wq_sb[kc][:, 128 * jh:128 * (jh + 1)],
                                xT_sb[kc][:, 512 * t4:512 * (t4 + 1)],
                                start=(kc == 0), stop=(kc == KC - 1))
                for jh in range(2):
                    for t4 in range(4):
                        nc.vector.tensor_scalar_add(
                            QT[jh][:, 512 * t4:512 * (t4 + 1)],
                            ps[jh][t4][:], bq_sb[jh][:])

            # ========= phase 2: V / attention / AG / projection =========
            with (
                tc.tile_pool(name="aps", bufs=1, space="PSUM") as aps,
                tc.tile_pool(name="ppool", bufs=1) as ppool,
                tc.tile_pool(name="npool", bufs=1) as npool,
                tc.tile_pool(name="ynp", bufs=1) as ynp,
                tc.tile_pool(name="yfp", bufs=1) as yfp,
                tc.tile_pool(name="otp", bufs=1) as otp,
            ):
                def emit_proj(tq):
                    yfs = []
                    for kd in range(KC):
                        yf = yfp.tile([128, 512], bf16, tag="yf", bufs=8,
                                      name=f"yf_{tq}_{kd}")
                        g2, p2 = divmod(kd, 2)
                        r0 = 256 * g2 + 128 * p2
                        dma_chunked(yf, yq_out[tq][r0:r0 + 128, :], 2)
                        yfs.append(yf)
                    for eh in range(2):
                        po = aps.tile([128, 512], f32, tag="po", bufs=1,
                                      name=f"po_{tq}_{eh}")
                        for kd in range(KC):
                            nc.tensor.matmul(
                                po[:],
                                wpT_sb[kd][:, 128 * eh:128 * (eh + 1)],
                                yfs[kd][:],
                                start=(kd == 0), stop=(kd == KC - 1))
                        ot = otp.tile([128, 512], f32, tag="ot", bufs=2,
                                      name=f"ot_{tq}_{eh}")
                        nc.vector.tensor_scalar_add(ot[:], po[:],
                                                    bp_sb[eh][:])
                        nc.sync.dma_start(
                            oT_d[128 * eh:128 * (eh + 1),
                                 512 * tq:512 * (tq + 1)], ot[:])

                for cq in range(NQC):
                    # K chunk cq (kc inner; x is resident by now) in a
                    # shared 's'-tag psum slot: attention cq only needs
                    # K^T[:, :512*(cq+1)], so K streams per chunk too
                    psk = aps.tile([128, 1024], f32, tag="s", bufs=2,
                                   name=f"psk_{cq}")
                    for kc in range(KC):
                        for jh in range(2):
                            nc.tensor.matmul(
                                psk[:, 512 * jh:512 * (jh + 1)],
                                wk_sb[kc][:, 128 * jh:128 * (jh + 1)],
                                xT_sb[kc][:, 512 * cq:512 * (cq + 1)],
                                start=(kc == 0), stop=(kc == KC - 1))
                    for jh in range(2):
                        nc.vector.tensor_scalar_add(
                            KT[jh][:, 512 * cq:512 * (cq + 1)],
                            psk[:, 512 * jh:512 * (jh + 1)], bk_sb[jh][:])

                    # V chunk cq: 4 k-tiles in one shared 's'-tag psum slot
                    psv = aps.tile([128, 1024], f32, tag="s", bufs=2,
                                   name=f"psv_{cq}")
                    for m in range(4):
                        mt = 4 * cq + m
                        for kc in range(KC):
                            nc.tensor.matmul(
                                psv[:, 256 * m:256 * (m + 1)],
                                xT_sb[kc][:, 128 * mt:128 * (mt + 1)],
                                wv_sb[kc][:],
                                start=(kc == 0), stop=(kc == KC - 1))
                    for m in range(4):
                        mt = 4 * cq + m
                        vv = V1[mt].rearrange("p (h x) -> p h x", h=HPG)
                        nc.vector.tensor_add(
                            vv[:, :, 0:HD],
                            psv[:, 256 * m:256 * (m + 1)].rearrange(
                                "p (h x) -> p h x", h=HPG),
                            bv_bc.rearrange("p (h x) -> p h x", h=HPG))
                        nc.sync.dma_start(vv[:, :, HD:HD + 1], ones_d[:])

                    for p in range(2):
                        yps = [aps.tile([HD + 1, 512], f32, tag="y",
                                        bufs=3, name=f"y_{cq}_{p}_{X}")
                               for X in range(2)]
                        nkt = 4 * (cq + 1)

                        def emit_S(kt):
                            qs = max(0, 128 * kt - 512 * cq)
                            S = aps.tile([128, 1024], f32, tag="s", bufs=2,
                                         name=f"s_{cq}_{p}_{kt}")
                            for X in range(2):
                                nc.tensor.matmul(
                                    S[:, 512 * X + qs:512 * (X + 1)],
                                    KT[p][64 * X:64 * (X + 1),
                                          128 * kt:128 * (kt + 1)],
                                    QT[p][64 * X:64 * (X + 1),
                                          512 * cq + qs:512 * (cq + 1)],
                                    start=True, stop=True)
                            return S

                        # S runs one kt ahead of AV so the exp latency is
                        # hidden behind the next S matmul pair
                        S_next = emit_S(0)
                        for kt in range(nkt):
                            qs = max(0, 128 * kt - 512 * cq)
                            S = S_next
                            Pt = ppool.tile([128, 1024], bf16, tag="p",
                                            bufs=4, name=f"p_{cq}_{p}_{kt}")
                            if qs == 0:
                                nc.scalar.activation(
                                    out=Pt[:], in_=S[:],
                                    func=Exp, scale=1.0 / math.sqrt(HD))
                            else:
                                nc.scalar.activation(
                                    out=Pt.rearrange("pp (x q) -> pp x q",
                                                     x=2)[:, :, qs:512],
                                    in_=S.rearrange("pp (x q) -> pp x q",
                                                    x=2)[:, :, qs:512],
                                    func=Exp, scale=1.0 / math.sqrt(HD))
                            if kt >= 4 * cq:  # diagonal block: causal mask
                                for X in range(2):
                                    nc.vector.tensor_mul(
                                        Pt[:, 512 * X + qs:512 * X + qs + 128],
                                        Pt[:, 512 * X + qs:512 * X + qs + 128],
                                        mask_sb[:])
                            if kt + 1 < nkt:
                                S_next = emit_S(kt + 1)
                            for X in range(2):
                                h = 2 * p + X
                                nc.tensor.matmul(
                                    yps[X][:, qs:512],
                                    V1[kt][:, (HD + 2) * h:
                                           (HD + 2) * h + HD + 1],
                                    Pt[:, 512 * X + qs:512 * (X + 1)],
                                    start=(kt == 0), stop=(kt == nkt - 1))

                        # normalization: psum drain + reciprocal on the ACT
                        # engine (idle at chunk boundaries; DVE is the
                        # congested one), broadcast on GpSimd, one DVE mul
                        yn = ynp.tile([128, 512], bf16, tag="yn", bufs=4,
                                      name=f"yn_{cq}_{p}")
                        for X in range(2):
                            ycp = npool.tile([HD, 512], bf16, tag="ycp",
                                             bufs=4, name=f"yc_{cq}_{p}_{X}")
                            nc.scalar.copy(ycp[:], yps[X][0:HD, :])
                            r1r = npool.tile([1, 512], f32, tag="r1r",
                                             bufs=4, name=f"r1r_{cq}_{p}_{X}")
                            nc.scalar.copy(r1r[:], yps[X][HD:HD + 1, :])
                            r1o = npool.tile([1, 512], f32, tag="r1o",
                                             bufs=4, name=f"r1o_{cq}_{p}_{X}")
                            nc.vector.reciprocal_approx_fast(r1o[:], r1r[:])
                            bcx = npool.tile([HD, 512], f32, tag="bc",
                                             bufs=4, name=f"bcx_{cq}_{p}_{X}")
                            nc.gpsimd.partition_broadcast(bcx[:], r1o[:])
                            nc.vector.tensor_mul(
                                yn[64 * X:64 * (X + 1), :],
                                ycp[:], bcx[:])
                        if cq == NQC - 1:
                            dma_chunked(yq3_in[p], yn, 2)
                            if p == 0:
                                nc.gpsimd.collective_compute(
                                    "AllGather", mybir.AluOpType.bypass,
                                    replica_groups=RG,
                                    ins=[yq3_in[0][:].opt()],
                                    outs=[yq3_out[0][:].opt()],
                                )
                        else:
                            dma_chunked(
                                yq_in[cq][128 * p:128 * (p + 1), :], yn, 2)
                    # proj one chunk behind: its AllGather completed during
                    # this chunk's attention.  Emitted BEFORE this chunk's
                    # AllGather trigger so the DRAM-pool reads don't order
                    # behind the newer collective.
                    if cq >= 1:
                        emit_proj(cq - 1)
                    if cq < NQC - 1:
                        nc.gpsimd.collective_compute(
                            "AllGather", mybir.AluOpType.bypass,
                            replica_groups=RG,
                            ins=[yq_in[cq][:].opt()],
                            outs=[yq_out[cq][:].opt()],
                        )
                    else:
                        nc.gpsimd.collective_compute(
                            "AllGather", mybir.AluOpType.bypass,
                            replica_groups=RG,
                            ins=[yq3_in[1][:].opt()],
                            outs=[yq3_out[1][:].opt()],
                        )

                # final projection in two pieces: the p0 contraction half
                # is emitted (and its AllGather finished) while p1's
                # gather is still in flight
                tq = NQC - 1
                yf3 = {}
                for phase, p2 in enumerate((0, 1)):
                    for g2 in range(G):
                        kd = 2 * g2 + p2
                        yf = yfp.tile([128, 512], bf16, tag="yf", bufs=8,
                                      name=f"yf3_{kd}")
                        dma_chunked(yf, yq3_out[p2][128 * g2:128 * (g2 + 1), :], 4)
                        yf3[kd] = yf
                    if phase == 0:
                        po3w = aps.tile([128, 1024], f32, tag="s", bufs=2,
                                        name="po3w")
                    for eh in range(2):
                        for i, g2 in enumerate(range(G)):
                            kd = 2 * g2 + p2
                            nc.tensor.matmul(
                                po3w[:, 512 * eh:512 * (eh + 1)],
                                wpT_sb[kd][:, 128 * eh:128 * (eh + 1)],
                                yf3[kd][:],
                                start=(phase == 0 and i == 0),
                                stop=(phase == 1 and i == G - 1))
                    if phase == 1:
                        for eh in range(2):
                            ot = otp.tile([128, 512], f32, tag="ot", bufs=2,
                                          name=f"ot3_{eh}")
                            nc.vector.tensor_scalar_add(
                                ot[:], po3w[:, 512 * eh:512 * (eh + 1)],
                                bp_sb[eh][:])
                            nc.sync.dma_start(
                                oT_d[128 * eh:128 * (eh + 1),
                                     512 * tq:512 * (tq + 1)], ot[:])

    nc.finalize()
    return nc


def _get_nc():
    if "nc" not in _NC_CACHE:
        _NC_CACHE["nc"] = _build()
    return _NC_CACHE["nc"]


def kernel(x, w_attn, b_attn, w_proj, b_proj):
    from concourse.bass_utils import run_bass_kernel_spmd

    x = np.asarray(x, dtype=np.float32)
    w_attn = np.asarray(w_attn, dtype=np.float32)
    b_attn = np.asarray(b_attn, dtype=np.float32)
    w_proj = np.asarray(w_proj, dtype=np.float32)
    b_proj = np.asarray(b_proj, dtype=np.float32)

    mask = np.triu(np.ones((128, 128), dtype=np.float32)).copy()

    in_maps = []
    for c in range(N_CORES):
        b, g = divmod(c, G)
        lo = DG * g
        wpT = np.ascontiguousarray(w_proj[lo:lo + DG, :].T)
        in_maps.append({
            "xT": np.ascontiguousarray(x[b].T).astype(ml_dtypes.bfloat16),
            "wq": np.ascontiguousarray(w_attn[lo:lo + DG, :].T).astype(ml_dtypes.bfloat16),
            "wk": np.ascontiguousarray(w_attn[C + lo:C + lo + DG, :].T).astype(ml_dtypes.bfloat16),
            "wv": np.ascontiguousarray(w_attn[2 * C + lo:2 * C + lo + DG, :].T).astype(ml_dtypes.bfloat16),
            "bq": np.ascontiguousarray(b_attn[lo:lo + DG].reshape(2, 128, 1)),
            "bk": np.ascontiguousarray(
                b_attn[C + lo:C + lo + DG].reshape(2, 128, 1)),
            "bv": np.ascontiguousarray(
                b_attn[2 * C + lo:2 * C + lo + DG].reshape(1, DG)),
            "wpTa": wpT.astype(ml_dtypes.bfloat16),
            "bp": np.ascontiguousarray(b_proj[lo:lo + DG].reshape(2, 128, 1)),
            "mask": mask.astype(ml_dtypes.bfloat16),
            "ones4": np.ones((128, HPG, 1), dtype=ml_dtypes.bfloat16),
        })

    global _last_in_maps
    _last_in_maps = in_maps

    nc = _get_nc()
    res = run_bass_kernel_spmd(nc, in_maps, list(range(N_CORES)))

    out = np.empty((B, T, C), dtype=np.float32)
    for c in range(N_CORES):
        b, g = divmod(c, G)
        out[b, :, DG * g:DG * (g + 1)] = res.results[c]["oT"].T
    return out
